# revision 76
# baseline (speedup 1.0000x reference)
"""Trainium2 Bass kernel for nn_C_Net_77807627534400 (sparse_attention).

Reference semantics: for each batch image and each class k in 1..11, the
per-class masked-normalized gray/rgb features form a correlation matrix,
softmax over the rgb-mask pixels, and a weighted mean of the rgb image is
written at the gray-mask pixels (if both masks have >= 2 pixels).

Every pixel belongs to exactly one class, so the attention is block-diagonal
over classes. The host gathers pixels by class into fixed padded tiles
(PG gray cols x PR rgb rows per class); each core processes 3 class slots of
one batch image (8 cores = 2 batches x 4 slots; the last slot of two cores is
an inert dummy). Per class slot, entirely on-chip:

    mean   = rowsum(f) * (1/cnt)          (DVE reduce, cnt from host metadata)
    sq     = (f - mean)^2                  (ACT Square with per-partition bias)
    ssq    = ones128^T @ sq                (PE; broadcast across partitions)
    rs     = exp(-0.5 * ln(ssq + eps))     (ACT; single act table has ln+exp)
    unit   = (f - mean) * rs               (DVE scalar_tensor_tensor fusion)
    corr   = unit_r^T @ unit_g             (PE, bf16, [PR, PG])
    E      = exp(corr - 1)                 (ACT; corr <= 1, no row-max needed)
    O4T    = E_chunk^T @ img4              (PE; gray pixels on partitions;
                                            img4 = [rgb image; mask row], so
                                            col 3 is the softmax denominator)
    out    = O4T[:, 0:3] * recip(O4T[:, 3])  (exact DVE reciprocal on [128,2])

Padded rgb pixels contribute nothing (img4 rows are zero there, including the
mask row that forms the softmax denominator); padded gray columns are
discarded by the host scatter. All matmuls run in bf16 (full PE rate; the
fp32 path is 4 cycles/row). The host does only layout work: gather by class
index, pad, transpose, dtype cast, and the final scatter into the -1 canvas.
"""

import numpy as np
from ml_dtypes import bfloat16

import concourse.bass as bass
import concourse.tile as tile
from concourse import mybir
from concourse.bass_utils import run_bass_kernel_spmd
from concourse.vector_clock import ScopedClock

B, C, H, W, NCH = 2, 256, 48, 48, 12
N = H * W            # 2304
PG = 232             # padded gray (output) pixels per class (max count 227)
PR = 232             # padded rgb (softmax) pixels per class
J1 = PR - 128        # second rgb partition chunk width (104)
I1 = PG - 128        # second gray partition chunk width
SLOTS = 3            # class slots per core
NCORES = 8
CLS_OF_SLOT = [[1, 2, 3], [4, 5, 6], [7, 8, 9], [10, 11, None]]
F32 = mybir.dt.float32
BF16 = mybir.dt.bfloat16
ALU = mybir.AluOpType
AF = mybir.ActivationFunctionType


class _TC(tile.TileContext):
    """Workaround: this walrus build rejects instructions carrying more than
    one sync-wait command. Split every multi-wait instruction into a chain of
    single-wait NOPs (same engine, program order preserved) followed by the
    original instruction holding the final wait."""

    def _add_instruction(self, inst):
        si = inst.sync_info
        if si is not None:
            waits = list(si.on_wait)
            if len(waits) > 1:
                nc = self.nc
                for w in waits[:-1]:
                    nop = mybir.InstNoOp(
                        name=nc.get_next_instruction_name(),
                        sync_info=mybir.SyncInfo(on_wait=[w], on_update=[]),
                        bass_nofuse=True,
                        engine=inst.engine,
                    )
                    super()._add_instruction(nop)
                si.on_wait = waits[-1:]
                inst.sync_info = si
        super()._add_instruction(inst)

    def _drain_and_barrier(self, tick_clock, wait_clock):
        nc = self.nc
        drain_inst = nc.sync.drain()
        wait_clock.add_sem_waits(
            drain_inst.ins, ScopedClock({None: tick_clock.global_clock})
        )
        si = drain_inst.ins.sync_info
        waits = list(si.on_wait) if si is not None else []
        if len(waits) > 1:
            si.on_wait = waits[:1]
            drain_inst.ins.sync_info = si
            for w in waits[1:]:
                extra = nc.sync.drain()
                extra.ins.sync_info = mybir.SyncInfo(on_wait=[w], on_update=[])

        nc.all_engine_barrier()
        assert self.sems is not None
        popped = nc._tile_sem_poison_stack.pop()
        assert popped is self._sem_poison
        nc.clear_and_free_semaphores(list(self.sems.allocated().values()))
        nc.all_engine_barrier()


def _build_nc():
    nc = bass.Bass(target_bir_lowering=False)

    # feat[s]: [128, (gc0|gc1|rc0|rc1), PR] bf16; the g/r halves are loaded
    # by separate DMAs (issued from different engine queues) so the gray side
    # can start as soon as the first half lands
    d_feat = nc.dram_tensor("feat", [SLOTS, 128, 4, PR], BF16,
                            kind="ExternalInput")
    # img4[s]: stationary [j, 4] per j-chunk: [128, slot, chunk, 4]
    d_img4 = nc.dram_tensor("img4", [128, SLOTS, 2, 4], BF16,
                            kind="ExternalInput")
    # consts cols: 0 = -1.0, 1 = 1e-12, 2 = 0.0 (ACT bias vectors)
    d_consts = nc.dram_tensor("consts", [128, 4], F32, kind="ExternalInput")
    # cvec cols 4s..4s+3 = (-1/cg, -1/cg, -1/cr, -1/cr) for slot s; bf16 so
    # the mean-scale and every downstream DVE op runs in 2x 16-bit mode
    d_cvec = nc.dram_tensor("cvec", [128, 12], BF16, kind="ExternalInput")
    # out[s]: [gray-pixel partition, i-chunk, rgb channel]
    d_out = nc.dram_tensor("outp", [SLOTS, 128, 2, 3], F32,
                           kind="ExternalOutput")

    with _TC(nc) as tc:
        with (
            tc.tile_pool(name="fixed", bufs=1) as fx,
            tc.tile_pool(name="feat", bufs=3) as fp,
            tc.tile_pool(name="work", bufs=3) as wk,
            tc.tile_pool(name="psS", bufs=3, space="PSUM") as psS,
            tc.tile_pool(name="psC", bufs=2, space="PSUM") as psC,
            tc.tile_pool(name="psO", bufs=2, space="PSUM") as psO,
            tc.tile_pool(name="psW", bufs=1, space="PSUM") as psW,
        ):
            # cvec/consts gate the first ops of every slot; the scalar-engine
            # HWDGE ring is measurably slow, so they ride the sync ring ahead
            # of the gray feature tiles. img4 (needed late) takes the slow one.
            cvec = fx.tile([128, 12], BF16)
            nc.sync.dma_start(cvec[:], d_cvec[:])
            consts = fx.tile([128, 4], F32)
            nc.sync.dma_start(consts[:], d_consts[:])
            img4 = fx.tile([128, SLOTS, 2, 4], BF16)
            nc.scalar.dma_start(img4[:], d_img4[:])
            ones128 = fx.tile([128, 128], BF16)
            nc.vector.memset(ones128[:], 1.0)

            # Dependency-free matmul stream that keeps the PE busy through
            # the startup DMA wait: the HAM clock gate only releases the
            # 2.4 GHz PE clock after ~3.4us of sustained activity, so without
            # this every (latency-critical) real matmul runs at 1.2 GHz.
            warm = fx.tile([128, 512], BF16)
            nc.gpsimd.memset(warm[:], 0.0)
            ps_warm = psW.tile([128, 512], F32)
            NWARM = 18
            for i in range(NWARM):
                nc.tensor.matmul(ps_warm[:], ones128[:], warm[:],
                                 start=(i == 0), stop=(i == NWARM - 1))

            st = [None] * SLOTS

            def front(s):
                f = fp.tile([128, 4, PR], BF16, tag="f", name=f"f{s}")
                nc.gpsimd.dma_start(f[:, 2:4, :], d_feat[s, :, 2:4, :])
                nc.sync.dma_start(f[:, 0:2, :], d_feat[s, :, 0:2, :])
                msum = wk.tile([128, 4], BF16, tag="msum", name=f"ms{s}")
                with nc.allow_low_precision("bf16 mean keeps DVE in 2x mode"):
                    nc.vector.tensor_reduce(msum[:, 2:4], f[:, 2:4, :],
                                            mybir.AxisListType.X, ALU.add)
                    nc.vector.tensor_reduce(msum[:, 0:2], f[:, 0:2, :],
                                            mybir.AxisListType.X, ALU.add)
                # tiny ops on the idle gpsimd queue: they fire right after the
                # reduce instead of queueing behind other slots' vector work,
                # so the ACT squares (which wait on negm) start ~1.5us earlier
                negm = wk.tile([128, 4], F32, tag="negm", bufs=3,
                               name=f"nm{s}")
                nc.gpsimd.tensor_mul(negm[:, 2:4], msum[:, 2:4],
                                     cvec[:, 4 * s + 2:4 * s + 4])
                nc.gpsimd.tensor_mul(negm[:, 0:2], msum[:, 0:2],
                                     cvec[:, 4 * s:4 * s + 2])

                # squares fused with the mean subtraction on ACT; layout
                # [c-chunk, side, PR] so ssq needs only 2 accumulating MMs
                sq = wk.tile([128, 2, 2, PR], BF16, tag="sq", name=f"sq{s}")
                for q in (2, 3, 0, 1):  # rgb first: it is the longer chain
                    nc.scalar.activation(sq[:, q % 2, q // 2, :], f[:, q, :],
                                         AF.Square, bias=negm[:, q:q + 1],
                                         scale=1.0)
                ps_ssq = psS.tile([128, 2, PR], F32, tag="ssq",
                                  name=f"ssq{s}")
                nc.tensor.matmul(ps_ssq[:], ones128[:], sq[:, 0, :, :],
                                 start=True, stop=False)
                nc.tensor.matmul(ps_ssq[:], ones128[:], sq[:, 1, :, :],
                                 start=False, stop=True)
                lnt = wk.tile([128, 2, PR], F32, tag="lnt", name=f"ln{s}")
                nc.scalar.activation(lnt[:], ps_ssq[:], AF.Ln,
                                     bias=consts[:, 1:2], scale=1.0)
                rs = wk.tile([128, 2, PR], BF16, tag="rs", bufs=3,
                             name=f"rs{s}")
                nc.scalar.activation(rs[:], lnt[:], AF.Exp,
                                     bias=consts[:, 2:3], scale=-0.5)
                st[s] = (f, negm, rs)

            def back(s):
                f, negm, rs = st[s]
                # c-chunk-major emission: the first corr accumulation only
                # needs the two c0 units, so the PE starts one chunk earlier
                unitr = wk.tile([128, 2, PR], BF16, tag="ur", name=f"ur{s}")
                unitg = wk.tile([128, 2, PG], BF16, tag="ug", name=f"ug{s}")
                for q in range(2):
                    nc.vector.scalar_tensor_tensor(
                        unitr[:, q, :], f[:, 2 + q, :], negm[:, 2 + q:3 + q],
                        rs[:, 1, :], ALU.add, ALU.mult)
                    nc.vector.scalar_tensor_tensor(
                        unitg[:, q, :], f[:, q, :], negm[:, q:q + 1],
                        rs[:, 0, :], ALU.add, ALU.mult)
                ps_corr = psC.tile([128, 2, PG], F32, tag="corr",
                                   name=f"corr{s}")
                for j, (j0, jw) in enumerate(((0, 128), (128, J1))):
                    nc.tensor.matmul(ps_corr[0:jw, j, :],
                                     unitr[:, 0, j0:j0 + jw],
                                     unitg[:, 0, :], start=True, stop=False)
                    nc.tensor.matmul(ps_corr[0:jw, j, :],
                                     unitr[:, 1, j0:j0 + jw],
                                     unitg[:, 1, :], start=False, stop=True)
                ee = wk.tile([128, 2, PG], BF16, tag="E", name=f"E{s}")
                if s == SLOTS - 1:
                    # last slot is the exposed tail: split the softmax exp per
                    # j-chunk so the first O4T accumulation overlaps the
                    # second exp instead of waiting for the batched op
                    nc.scalar.activation(ee[:, 0, :], ps_corr[:, 0, :],
                                         AF.Exp, bias=consts[:, 0:1],
                                         scale=1.0)
                    nc.scalar.activation(ee[:, 1, :], ps_corr[:, 1, :],
                                         AF.Exp, bias=consts[:, 0:1],
                                         scale=1.0)
                else:
                    nc.scalar.activation(ee[:], ps_corr[:], AF.Exp,
                                         bias=consts[:, 0:1], scale=1.0)
                ps_o4t = psO.tile([128, 2, 4], F32, tag="O4", name=f"O4{s}")
                for q, (i0, iw) in enumerate(((0, 128), (128, I1))):
                    nc.tensor.matmul(ps_o4t[0:iw, q, :],
                                     ee[0:128, 0, i0:i0 + iw],
                                     img4[:, s, 0, :], start=True, stop=False)
                    nc.tensor.matmul(ps_o4t[0:iw, q, :],
                                     ee[0:J1, 1, i0:i0 + iw],
                                     img4[0:J1, s, 1, :],
                                     start=False, stop=True)
                rcpt = wk.tile([128, 2], F32, tag="rcpt", name=f"rc{s}")
                nc.vector.reciprocal(
                    rcpt[:], ps_o4t[:, :, 3:4].rearrange("p a b -> p (a b)"))
                rest = wk.tile([128, 2, 3], F32, tag="res", name=f"res{s}")
                for q in range(2):
                    nc.vector.tensor_scalar(rest[:, q, :], ps_o4t[:, q, 0:3],
                                            rcpt[:, q:q + 1], None, ALU.mult)
                nc.sync.dma_start(d_out[s], rest[:])

            # software pipeline across class slots (the tile scheduler
            # further reorders per-engine streams globally)
            front(0)
            front(1)
            front(2)
            back(0)
            back(1)
            back(2)

    return nc


_NC_CACHE = None


def _get_nc():
    global _NC_CACHE
    if _NC_CACHE is None:
        _NC_CACHE = _build_nc()
    return _NC_CACHE


def build_in_maps(gray_feature, rgb_feature, rgb_image, gray_label, rgb_label):
    gf_all = np.ascontiguousarray(gray_feature, np.float32).reshape(B, C, N)
    rf_all = np.ascontiguousarray(rgb_feature, np.float32).reshape(B, C, N)
    img_all = np.ascontiguousarray(rgb_image, np.float32).reshape(B, 3, N)
    gl_all = np.asarray(gray_label, np.float32).reshape(B, NCH, N) > 0.5
    rl_all = np.asarray(rgb_label, np.float32).reshape(B, NCH, N) > 0.5

    in_maps = []
    meta = []  # per core: list of (class k or None, Ig, valid)
    for core in range(NCORES):
        b, q = divmod(core, 4)
        feat = np.zeros((SLOTS, 128, 4, PR), bfloat16)
        img4 = np.zeros((128, SLOTS, 2, 4), bfloat16)
        consts = np.zeros((128, 4), np.float32)
        consts[:, 0] = -1.0
        consts[:, 1] = 1e-12
        cvec = np.zeros((128, 12), bfloat16)
        core_meta = []
        for s, k in enumerate(CLS_OF_SLOT[q]):
            if k is None:
                cvec[:, 4 * s:4 * s + 4] = -1.0
                img4[:, s, :, 3] = 1.0  # keep the denominator away from 0
                core_meta.append((None, None, False))
                continue
            ig = np.nonzero(gl_all[b, k])[0]
            ir = np.nonzero(rl_all[b, k])[0]
            ng, nr = len(ig), len(ir)
            assert ng <= PG and nr <= PR, (ng, nr)
            fb = np.zeros((4, 128, PR), np.float32)
            fb[0:2, :, :ng] = gf_all[b][:, ig].reshape(2, 128, ng)
            fb[2:4, :, :nr] = rf_all[b][:, ir].reshape(2, 128, nr)
            feat[s] = fb.transpose(1, 0, 2)
            i4 = np.zeros((4, 256), np.float32)
            i4[0:3, :nr] = img_all[b][:, ir]
            i4[3, :nr] = 1.0
            img4[:, s, :, :] = i4.reshape(4, 2, 128).transpose(2, 1, 0)
            cvec[:, 4 * s:4 * s + 2] = -1.0 / max(ng, 1)
            cvec[:, 4 * s + 2:4 * s + 4] = -1.0 / max(nr, 1)
            core_meta.append((k, ig, ng > 1 and nr > 1))
        in_maps.append({"feat": feat, "img4": img4,
                        "consts": consts, "cvec": cvec})
        meta.append(core_meta)
    return in_maps, meta


def kernel(gray_feature, rgb_feature, rgb_image, gray_label, rgb_label):
    in_maps, meta = build_in_maps(gray_feature, rgb_feature, rgb_image,
                                  gray_label, rgb_label)
    res = run_bass_kernel_spmd(_get_nc(), in_maps, list(range(NCORES)))

    canvas = np.full((B, 3, N), -1.0, np.float32)
    for core in range(NCORES):
        b = core // 4
        out = res.results[core]["outp"]  # [SLOTS, 128, 2, 3]
        for s, (k, ig, valid) in enumerate(meta[core]):
            if k is None or not valid:
                continue
            pix = out[s].transpose(1, 0, 2).reshape(256, 3)
            canvas[b][:, ig] = pix[:len(ig)].T
    return canvas.reshape(B, 3, H, W)
